# revision 2
# baseline (speedup 1.0000x reference)
"""Chamfer + KL loss on 8 Trainium2 NeuronCores — v3 (single-pass).

Key idea vs v2: both squared norms are folded INTO the matmul (K=13
hi/lo bf16 rows), so each PSUM tile holds the full distance matrix
D = ||x||^2 + ||y||^2 - 2xy >= 0.  One matmul pass then serves BOTH
Chamfer orientations:

  - row mins (per stationary point): one DVE tensor_scalar per
    [128,2048] PSUM tile with op0=min(+inf) (a copy) and accum_out =
    free-dim min.  The same instruction also materializes the tile as
    bf16 in SBUF.
  - col mins (per moving point): elementwise bf16 min of the SBUF
    copies into V[b] ([128,4096]), accumulated across the 32
    stationary blocks (DVE tensor_tensor 2x mode, optionally GPSIMD
    for a subset).  The final min across the 128 partitions of V is
    done on the host (O(B*M) work, like the R4 combine).

This halves the drained PSUM volume vs v2's two-pass scheme; the DVE
is the bottleneck engine at ~1 column/cycle (no fast modes exist for
reduce from PSUM on TRN2).
"""
import sys

sys.path.insert(0, "/opt/trn_rl_repo")

import numpy as np
import jax
from jax.sharding import Mesh, PartitionSpec
from jax.experimental.shard_map import shard_map

import concourse.bacc as bacc
import concourse.tile as tile
import concourse.mybir as mybir
from concourse.bass2jax import (
    _bass_exec_p,
    install_neuronx_cc_hook,
    partition_id_tensor,
)

F32 = mybir.dt.float32
BF16 = mybir.dt.bfloat16
MN = mybir.AluOpType.min
X = mybir.AxisListType.X

N_CORES = 8
B_PER_CORE = 2
NPTS = 4096
NBLK = 32           # stationary 128-blocks per batch
HW = 2048           # half-unit width (PSUM tile = [128, HW] f32 = 4 banks)
NH = NPTS // HW     # 2 half-units per unit
N_UNITS = B_PER_CORE * NBLK * NH   # 128 half-unit tiles per core
Z = 128
BIG = 3e38

# number of half-unit tiles (of N_UNITS) whose V-accumulation runs on
# GPSIMD instead of DVE
N_GPS = 0


def _build_nc(rep=1, n_gps=None, mm_only=False):
    n_gps = N_GPS if n_gps is None else n_gps
    # evenly spread GPS-accum tiles among the half-units
    gps_set = set()
    if n_gps:
        step = N_UNITS / n_gps
        for k in range(n_gps):
            gps_set.add(min(N_UNITS - 1, int(k * step)))

    nc = bacc.Bacc("TRN2", target_bir_lowering=False, debug=False)
    gts_d = nc.dram_tensor("gts_c", [B_PER_CORE, 3, NPTS], F32, kind="ExternalInput")
    preds_d = nc.dram_tensor("preds_c", [B_PER_CORE, 3, NPTS], F32, kind="ExternalInput")
    mu_d = nc.dram_tensor("mu_c", [B_PER_CORE, Z], F32, kind="ExternalInput")
    lv_d = nc.dram_tensor("logvar_c", [B_PER_CORE, Z], F32, kind="ExternalInput")
    # per-half-unit row mins: [128, 2 * NBLK * NH] (batch-major columns)
    R_out = nc.dram_tensor("R_out", [128, B_PER_CORE * NBLK * NH], F32,
                           kind="ExternalOutput")
    V_out = nc.dram_tensor("V_out", [B_PER_CORE, 128, NPTS], BF16, kind="ExternalOutput")
    kl_out = nc.dram_tensor("kl_out", [B_PER_CORE, 1], F32, kind="ExternalOutput")

    src_d = {0: gts_d, 1: preds_d}  # 0 = stationary cloud, 1 = moving cloud

    with tile.TileContext(nc) as tc:
        with (
            tc.tile_pool(name="sb", bufs=1) as sb,
            tc.tile_pool(name="dramp", bufs=8, space="DRAM") as dramp,
            tc.tile_pool(name="ps", bufs=2, space="PSUM") as ps,
            tc.tile_pool(name="tb", bufs=2) as tb,
        ):
            # ---- staging: all coordinate matrices as [12, 4096] f32
            stage = sb.tile([12, NPTS], F32, tag="stage")
            for b in range(B_PER_CORE):
                nc.sync.dma_start(stage[6 * b + 0 : 6 * b + 3, :], gts_d[b])
                nc.sync.dma_start(stage[6 * b + 3 : 6 * b + 6, :], preds_d[b])

            def rows(b, which):
                r0 = 6 * b + 3 * which
                return r0, r0 + 3

            # ---- split forms (all [12, 4096]); x-side carries the -2
            m2 = sb.tile([12, NPTS], F32, tag="m2")
            nc.vector.tensor_scalar_mul(m2[:], stage[:], -2.0)
            sh = sb.tile([12, NPTS], BF16, tag="sh")
            nc.vector.tensor_copy(sh[:], m2[:])
            sl = sb.tile([12, NPTS], BF16, tag="sl")
            nc.vector.tensor_tensor(out=sl[:], in0=m2[:], in1=sh[:], op=mybir.AluOpType.subtract)
            mh = sb.tile([12, NPTS], BF16, tag="mh")
            nc.vector.tensor_copy(mh[:], stage[:])
            ml = sb.tile([12, NPTS], BF16, tag="ml")
            nc.vector.tensor_tensor(out=ml[:], in0=stage[:], in1=mh[:], op=mybir.AluOpType.subtract)

            ones2 = sb.tile([2, NPTS], BF16, tag="ones2")
            nc.vector.memset(ones2[:], 1.0)

            # ---- squared norms -> bf16 hi/lo rows [1, 4096] via DRAM bounce
            norm_rows = {}
            for b in range(B_PER_CORE):
                for which in range(2):
                    d0 = src_d[which]
                    ct = []
                    for d in range(3):
                        cd = sb.tile([128, 32], F32, tag=f"c{d}")
                        nc.sync.dma_start(
                            cd[:], d0[b, d].rearrange("(p c) -> p c", p=128)
                        )
                        ct.append(cd)
                    s0 = sb.tile([128, 32], F32, tag="s0")
                    t0 = sb.tile([128, 32], F32, tag="t0")
                    nc.vector.tensor_tensor(out=s0[:], in0=ct[0][:], in1=ct[0][:], op=mybir.AluOpType.mult)
                    nc.vector.tensor_tensor(out=t0[:], in0=ct[1][:], in1=ct[1][:], op=mybir.AluOpType.mult)
                    nc.vector.tensor_tensor(out=s0[:], in0=s0[:], in1=t0[:], op=mybir.AluOpType.add)
                    nc.vector.tensor_tensor(out=t0[:], in0=ct[2][:], in1=ct[2][:], op=mybir.AluOpType.mult)
                    nc.vector.tensor_tensor(out=s0[:], in0=s0[:], in1=t0[:], op=mybir.AluOpType.add)
                    rh = sb.tile([128, 32], BF16, tag="rh")
                    nc.vector.tensor_copy(rh[:], s0[:])
                    rl = sb.tile([128, 32], BF16, tag="rl")
                    nc.vector.tensor_tensor(out=rl[:], in0=s0[:], in1=rh[:], op=mybir.AluOpType.subtract)
                    bh = dramp.tile([128, 32], BF16, tag="bh")
                    bl = dramp.tile([128, 32], BF16, tag="bl")
                    nc.sync.dma_start(bh[:], rh[:])
                    nc.sync.dma_start(bl[:], rl[:])
                    norm_rows[(b, which)] = (bh, bl)

            # ---- assemble stationary / moving tensors [128, 4096] bf16, K=13
            # stationary rows: sh(3) sh(3) sl(3) rxh rxl 1 1
            # moving rows:     mh(3) ml(3) mh(3) 1   1   ryh ryl
            stat_all = {}
            mov_all = {}
            for b in range(B_PER_CORE):
                r0, r1 = rows(b, 0)   # stationary = gts
                m0, m1 = rows(b, 1)   # moving = preds
                sxh, sxl = norm_rows[(b, 0)]
                syh, syl = norm_rows[(b, 1)]
                sa = sb.tile([128, NPTS], BF16, tag=f"stat{b}")
                nc.sync.dma_start(sa[0:3, :], sh[r0:r1, :])
                nc.sync.dma_start(sa[3:6, :], sh[r0:r1, :])
                nc.sync.dma_start(sa[6:9, :], sl[r0:r1, :])
                nc.sync.dma_start(sa[9:10, :], sxh[:].rearrange("p c -> (p c)")[None, :])
                nc.sync.dma_start(sa[10:11, :], sxl[:].rearrange("p c -> (p c)")[None, :])
                nc.sync.dma_start(sa[11:13, :], ones2[:])
                ma = sb.tile([128, NPTS], BF16, tag=f"mov{b}")
                nc.sync.dma_start(ma[0:3, :], mh[m0:m1, :])
                nc.sync.dma_start(ma[3:6, :], ml[m0:m1, :])
                nc.sync.dma_start(ma[6:9, :], mh[m0:m1, :])
                nc.sync.dma_start(ma[9:11, :], ones2[:])
                nc.sync.dma_start(ma[11:12, :], syh[:].rearrange("p c -> (p c)")[None, :])
                nc.sync.dma_start(ma[12:13, :], syl[:].rearrange("p c -> (p c)")[None, :])
                for g in range(1, 4):
                    nc.sync.dma_start(sa[32 * g : 32 * g + 13, :], sa[0:13, :])
                    nc.sync.dma_start(ma[32 * g : 32 * g + 13, :], ma[0:13, :])
                stat_all[b] = sa
                mov_all[b] = ma

            # ---- KL pieces: t_b = sum_z (logvar - mu^2 - exp(logvar))
            mu_t = sb.tile([B_PER_CORE, Z], F32, tag="mu_t")
            lv_t = sb.tile([B_PER_CORE, Z], F32, tag="lv_t")
            nc.sync.dma_start(mu_t[:], mu_d[:])
            nc.sync.dma_start(lv_t[:], lv_d[:])
            msq = sb.tile([B_PER_CORE, Z], F32, tag="msq")
            nc.vector.tensor_tensor(out=msq[:], in0=mu_t[:], in1=mu_t[:], op=mybir.AluOpType.mult)
            ex = sb.tile([B_PER_CORE, Z], F32, tag="ex")
            nc.scalar.activation(ex[:], lv_t[:], mybir.ActivationFunctionType.Exp)
            kt = sb.tile([B_PER_CORE, Z], F32, tag="kt")
            nc.vector.tensor_tensor(out=kt[:], in0=lv_t[:], in1=msq[:], op=mybir.AluOpType.subtract)
            nc.vector.tensor_tensor(out=kt[:], in0=kt[:], in1=ex[:], op=mybir.AluOpType.subtract)
            kl_t = sb.tile([B_PER_CORE, 1], F32, tag="kl_t")
            nc.vector.tensor_reduce(kl_t[:], kt[:], axis=X, op=mybir.AluOpType.add)
            nc.sync.dma_start(kl_out[:], kl_t[:])

            # ---- accumulators
            Racc = sb.tile([128, B_PER_CORE * NBLK * NH], F32, tag="Racc")
            nc.vector.memset(Racc[:], BIG)
            V = {}
            for b in range(B_PER_CORE):
                vb = sb.tile([128, NPTS], BF16, tag=f"V{b}", name=f"V{b}")
                nc.vector.memset(vb[:], BIG)
                V[b] = vb

            # ---- main loop
            for _rep in range(rep):
                for b in range(B_PER_CORE):
                    sa = stat_all[b]
                    ma = mov_all[b]
                    for i in range(NBLK):
                        T = tb.tile([128, NPTS], BF16, tag="T")
                        for jj in range(NH):
                            Pe = ps.tile([128, HW], F32, tag="Pe")
                            for s in range(4):
                                g = 32 * ((s + 2 * jj + i) % 4)
                                nc.tensor.matmul(
                                    Pe[:, s * 512 : (s + 1) * 512],
                                    sa[g : g + 13, i * 128 : (i + 1) * 128],
                                    ma[g : g + 13, jj * HW + s * 512 : jj * HW + (s + 1) * 512],
                                    start=True, stop=True,
                                    tile_position=(g, 0),
                                )
                            if mm_only:
                                continue
                            col = (b * NBLK + i) * NH + jj
                            # i=0: the tile IS the initial V (skip the TT)
                            dest = V[b][:, jj * HW : (jj + 1) * HW] if i == 0                                 else T[:, jj * HW : (jj + 1) * HW]
                            nc.vector.tensor_scalar(
                                out=dest,
                                in0=Pe[:],
                                scalar1=BIG, scalar2=BIG,
                                op0=MN, op1=MN,
                                accum_out=Racc[:, col : col + 1],
                            )
                        if mm_only or i == 0:
                            continue
                        nc.vector.tensor_tensor(
                            out=V[b][:], in0=T[:], in1=V[b][:], op=MN)

            nc.sync.dma_start(R_out[:], Racc[:])
            for b in range(B_PER_CORE):
                nc.sync.dma_start(V_out[b], V[b][:])
    nc.compile()
    return nc


class _Runner:
    def __init__(self, nc, n_cores):
        install_neuronx_cc_hook()
        self.n_cores = n_cores
        partition_name = nc.partition_id_tensor.name if nc.partition_id_tensor else None
        in_names, out_names, out_avals, zero_outs = [], [], [], []
        for alloc in nc.m.functions[0].allocations:
            if not isinstance(alloc, mybir.MemoryLocationSet):
                continue
            name = alloc.memorylocations[0].name
            if alloc.kind == "ExternalInput":
                if name != partition_name:
                    in_names.append(name)
            elif alloc.kind == "ExternalOutput":
                out_names.append(name)
                shape = tuple(alloc.tensor_shape)
                dtype = mybir.dt.np(alloc.dtype)
                out_avals.append(jax.core.ShapedArray(shape, dtype))
                zero_outs.append(np.zeros(shape, dtype))
        self.in_names, self.out_names = in_names, out_names
        self.out_avals, self.zero_outs = out_avals, zero_outs
        n_params, n_outs = len(in_names), len(out_names)
        self.n_params = n_params
        all_in = list(in_names) + list(out_names)
        if partition_name is not None:
            all_in.append(partition_name)
        donate = tuple(range(n_params, n_params + n_outs))

        def _body(*args):
            operands = list(args)
            if partition_name is not None:
                operands.append(partition_id_tensor())
            return tuple(
                _bass_exec_p.bind(
                    *operands,
                    out_avals=tuple(out_avals),
                    in_names=tuple(all_in),
                    out_names=tuple(out_names),
                    lowering_input_output_aliases=(),
                    sim_require_finite=True,
                    sim_require_nnan=True,
                    nc=nc,
                )
            )

        devices = jax.devices()[:n_cores]
        mesh = Mesh(np.asarray(devices), ("core",))
        in_specs = (PartitionSpec("core"),) * (n_params + n_outs)
        out_specs = (PartitionSpec("core"),) * n_outs
        self.fn = jax.jit(
            shard_map(_body, mesh=mesh, in_specs=in_specs, out_specs=out_specs,
                      check_rep=False),
            donate_argnums=donate,
            keep_unused=True,
        )

    def run(self, in_maps):
        n = self.n_cores
        per_core = [[np.asarray(m[k]) for k in self.in_names] for m in in_maps]
        concat_in = [
            np.concatenate([per_core[c][i] for c in range(n)], axis=0)
            for i in range(self.n_params)
        ]
        concat_zeros = [
            np.zeros((n * z.shape[0], *z.shape[1:]), z.dtype) for z in self.zero_outs
        ]
        outs = self.fn(*concat_in, *concat_zeros)
        return [
            {
                k: np.asarray(outs[i]).reshape(n, *self.out_avals[i].shape)[c]
                for i, k in enumerate(self.out_names)
            }
            for c in range(n)
        ]


_CACHE = {}


def _get_runner():
    if "r" not in _CACHE:
        nc = _build_nc()
        _CACHE["r"] = _Runner(nc, N_CORES)
    return _CACHE["r"]


def _combine(outs):
    """Host-side combine of per-core partials (float64)."""
    total = 0.0
    for c in range(N_CORES):
        R = outs[c]["R_out"].astype(np.float64)     # [128, B*NBLK*NH]
        Vv = outs[c]["V_out"].astype(np.float64)    # [B, 128, 4096]
        kl_t = outs[c]["kl_out"].astype(np.float64)  # [B, 1]
        # loss_2: sum over stationary points of row mins
        Rr = R.reshape(128, B_PER_CORE, NBLK, NH)
        total += Rr.min(axis=3).sum()
        # loss_1: per moving point, min across all stationary partitions
        total += Vv.min(axis=1).sum()
        for b in range(B_PER_CORE):
            total += -0.5 * (Z + kl_t[b, 0])
    return total


def kernel(preds, gts, mu, logvar):
    preds = np.asarray(preds, np.float32)
    gts = np.asarray(gts, np.float32)
    mu = np.asarray(mu, np.float32)
    logvar = np.asarray(logvar, np.float32)
    runner = _get_runner()
    in_maps = []
    for c in range(N_CORES):
        sl = slice(B_PER_CORE * c, B_PER_CORE * (c + 1))
        in_maps.append(
            {
                "gts_c": gts[sl],
                "preds_c": preds[sl],
                "mu_c": mu[sl],
                "logvar_c": logvar[sl],
            }
        )
    outs = runner.run(in_maps)
    return np.float32(_combine(outs))
